# revision 1
# baseline (speedup 1.0000x reference)
"""Rank-1 triangular attention via Taylor moments, v2.

Math (per batch): k = x @ wk, q = x @ wq, c = q/32; column-softmax over
i <= j of exp(k_i c_j); out[j] = sum_i a[i,j] f[i].  |k c| <= ~0.21 so
exp(k_i c_j) = sum_p (k_i^p/p!) c_j^p (5 terms, err ~3e-6).  Off-diagonal
block contributions become moment prefix sums C_p[d] = sum_i k_i^p f[i,d];
the diagonal 128x128 block uses exact exp with a triangular mask.

v2 changes vs v1:
  - host casts: x -> fp8 (transposed to [D,N]), f -> bf16, out -> bf16.
    HBM traffic 24MB -> 10MB per core (rel-err budget is 2e-2; this lands
    well inside it).
  - k,q computed on the tensor engine from the transposed x (w stationary,
    x streaming), replacing 32 slow DVE scalar_tensor_tensor passes.
  - moments live in ONE psum bank as 4 column-groups of 5 partitions
    ([32q..32q+4, 0:256] holds d-quarter q); the 4 moment matmuls and the
    4 V^T C matmuls use PE row/col tiling and run concurrently; snapshots
    copy [101,257] instead of [5,1025] (4x fewer elems/lane).
  - PE warm-up matmuls at t=0 so the HAM clock-gate is released before the
    real matmul stream starts.
"""

import sys

sys.path.insert(0, "/opt/trn_rl_repo")

import numpy as np
import ml_dtypes

B, N, D = 8, 2048, 1024
P = 128
NT = N // P       # 16 i/j tiles
PD = 5            # Taylor terms p=0..4
QD = D // 4       # 256, d-quarter per column-group
HALF = D // 2     # 512
NCH = D // P      # 8 d-chunks for the k/q matmuls

_CACHE = {}


def _patch_compiler_flags():
    from concourse import compiler_utils as cu

    flags = [f.replace("--enable-ldw-opt=false", "--enable-ldw-opt=true")
             for f in cu.get_compiler_flags()]
    cu.set_compiler_flags(flags)


def _build():
    import concourse.bacc as bacc
    import concourse.mybir as mybir
    from concourse.tile import TileContext
    from concourse.masks import make_identity, make_upper_triangular

    _patch_compiler_flags()

    dt = mybir.dt
    f32 = dt.float32
    bf16 = dt.bfloat16
    fp8 = dt.float8e4
    AF = mybir.ActivationFunctionType
    ALU = mybir.AluOpType

    nc = bacc.Bacc(None, target_bir_lowering=False)
    xt_ext = nc.declare_dram_parameter("xt", [D, N], fp8, isOutput=False)
    f_ext = nc.declare_dram_parameter("f", [N, D], bf16, isOutput=False)
    w_ext = nc.declare_dram_parameter("w", [D, 32], fp8, isOutput=False)
    out_ext = nc.declare_dram_parameter("out", [N, D], bf16, isOutput=True)

    with TileContext(nc) as tc:
        with (
            tc.tile_pool(name="const", bufs=1) as cpool,
            tc.tile_pool(name="fin", bufs=3) as fpool,
            tc.tile_pool(name="sd", bufs=3) as sdpool,
            tc.tile_pool(name="outsb", bufs=2) as opool,
            tc.tile_pool(name="rz", bufs=3) as rzpool,
            tc.tile_pool(name="ps_C", bufs=1, space="PSUM") as ps_C_pool,
            tc.tile_pool(name="ps_out", bufs=3, space="PSUM") as ps_out_pool,
            tc.tile_pool(name="ps_kq", bufs=1, space="PSUM") as ps_kq_pool,
            tc.tile_pool(name="ps_vt", bufs=1, space="PSUM") as ps_vt_pool,
            tc.tile_pool(name="ps_small", bufs=2, space="PSUM") as ps_small_pool,
        ):
            # ---------------- constants ----------------
            ident_bf = cpool.tile([P, P], bf16, tag="ident_bf")
            make_identity(nc, ident_bf[:])
            triu = cpool.tile([P, P], bf16, tag="triu")
            make_upper_triangular(nc, triu[:], val=1.0, diag=True)
            ones_col = cpool.tile([P, 1], bf16, tag="ones_col")
            nc.gpsimd.memset(ones_col[:], 1.0)

            # persistent moment accumulator, one bank:
            # [32q + p, 0:256] = C_p for d-quarter q; [0:5, 256] = H_p
            psum_C = ps_C_pool.tile([3 * 32 + PD, QD + 1], f32, tag="psC",
                                    name="psC")
            nc.vector.memset(psum_C[:], 0.0)

            # Persistent round-robin snapshot/v4 buffers, zeroed once: the
            # per-tile writes only touch the same group-rows/column-blocks,
            # so the zero gaps (which make the plain K=128 V@C matmuls
            # select the right quarter) persist across reuse.
            # persistent PE-transpose target: group rows are rewritten every
            # tile, gap rows stay zero from this one memset
            vt_ps = ps_vt_pool.tile([3 * 32 + PD, P], f32, tag="vtps",
                                    name="vtps")
            nc.vector.memset(vt_ps[:], 0.0)

            c_sbs = []
            v4s = []
            for i in range(3):
                zt = cpool.tile([P, D + 1], bf16, tag=f"csnapP{i}",
                                name=f"csnapP{i}")
                nc.vector.memset(zt[:], 0.0)
                c_sbs.append(zt)
                zv = cpool.tile([P, P], bf16, tag=f"v4P{i}", name=f"v4P{i}")
                nc.vector.memset(zv[:], 0.0)
                v4s.append(zv)

            # PE warm-up: ~8 x 512-col matmuls of garbage keeps the HAM
            # activity window busy so the real stream runs at 2.4 GHz.
            warm_ps = ps_out_pool.tile([P, HALF], f32, tag="out_ps",
                                       name="warm")
            for _ in range(32):
                nc.tensor.matmul(
                    warm_ps[:, 0:P], lhsT=ident_bf[:], rhs=triu[:],
                    start=True, stop=True, skip_group_check=True,
                )

            # ---------------- stage A: k,q ----------------
            w_sb = cpool.tile([P, NCH, 32], fp8, tag="w_sb")
            nc.sync.dma_start(
                out=w_sb[:], in_=w_ext.rearrange("(c p) m -> p c m", p=P))
            xt_sb = cpool.tile([P, NCH, N], fp8, tag="xt_sb")
            for h in range(4):
                nc.sync.dma_start(
                    out=xt_sb[:, 2 * h:2 * h + 2, :],
                    in_=xt_ext[h * 2 * P:(h + 1) * 2 * P, :].rearrange(
                        "(c p) n -> p c n", p=P),
                )

            # kq_sb rows: 0 = 32k, 1 = 32q (host pre-scales w by 32 to keep
            # fp8 weights in the normal range)
            kq_sb = cpool.tile([32, N], bf16, tag="kq_sb")
            for c in range(4):
                ps = ps_kq_pool.tile([32, HALF], f32, tag="kq_ps",
                                     name=f"kq_ps{c}")
                for ch in range(NCH):
                    nc.tensor.matmul(
                        ps[:], lhsT=w_sb[:, ch, :],
                        rhs=xt_sb[:, ch, c * HALF:(c + 1) * HALF],
                        start=(ch == 0), stop=(ch == NCH - 1),
                    )
                nc.scalar.copy(kq_sb[:, c * HALF:(c + 1) * HALF], ps[:])

            # per-tile k/c columns: PE transpose of [32,128] slices
            kc_all = cpool.tile([P, NT, 2], f32, tag="kc_all")
            for t in range(NT):
                kc_ps = ps_small_pool.tile([P, P + 1 + P], f32, tag="smalls",
                                           name=f"kc_ps{t}")
                nc.tensor.matmul(
                    kc_ps[:, 0:32], lhsT=kq_sb[:, t * P:(t + 1) * P],
                    rhs=ident_bf[0:32, 0:32], start=True, stop=True,
                )
                nc.vector.tensor_copy(kc_all[:, t, :], kc_ps[:, 0:2])

            # bulk power tables over all tiles at once ([128,16] ops)
            kT = cpool.tile([P, NT], f32, tag="kT")   # true k (for exp scale)
            nc.vector.tensor_scalar_mul(kT[:], kc_all[:, :, 0], 1.0 / 32.0)
            kp_all = cpool.tile([P, NT, PD], bf16, tag="kp_all")
            cp_all = cpool.tile([P, NT, PD], bf16, tag="cp_all")
            nc.gpsimd.memset(kp_all[:, :, 0:1], 1.0)
            nc.gpsimd.memset(cp_all[:, :, 0:1], 1.0)
            nc.vector.tensor_copy(kp_all[:, :, 1], kT[:])
            nc.vector.tensor_scalar_mul(cp_all[:, :, 1], kc_all[:, :, 1],
                                        1.0 / 1024.0)
            # kp: k^p/p! ; cp: c^p (factorials folded into the k side)
            nc.vector.scalar_tensor_tensor(
                out=kp_all[:, :, 2], in0=kp_all[:, :, 1], scalar=0.5,
                in1=kp_all[:, :, 1], op0=ALU.mult, op1=ALU.mult)
            nc.vector.scalar_tensor_tensor(
                out=kp_all[:, :, 3], in0=kp_all[:, :, 2], scalar=1.0 / 3.0,
                in1=kp_all[:, :, 1], op0=ALU.mult, op1=ALU.mult)
            nc.vector.scalar_tensor_tensor(
                out=kp_all[:, :, 4], in0=kp_all[:, :, 3], scalar=0.25,
                in1=kp_all[:, :, 1], op0=ALU.mult, op1=ALU.mult)
            nc.vector.scalar_tensor_tensor(
                out=cp_all[:, :, 2], in0=cp_all[:, :, 1], scalar=1.0,
                in1=cp_all[:, :, 1], op0=ALU.mult, op1=ALU.mult)
            nc.vector.scalar_tensor_tensor(
                out=cp_all[:, :, 3], in0=cp_all[:, :, 2], scalar=1.0,
                in1=cp_all[:, :, 1], op0=ALU.mult, op1=ALU.mult)
            nc.vector.scalar_tensor_tensor(
                out=cp_all[:, :, 4], in0=cp_all[:, :, 3], scalar=1.0,
                in1=cp_all[:, :, 1], op0=ALU.mult, op1=ALU.mult)

            # ---------------- main loop ----------------
            fq = None
            o_sb = None
            for t in range(NT):
                if t % 4 == 0:
                    g = t // 4
                    fq = fpool.tile([P, 4, D], bf16, tag="fq", name=f"fq{g}")
                    # floor f loads at ~5.5us so the xT load (which gates all
                    # compute) gets the HBM bandwidth to itself first
                    with tc.tile_wait_until(0.0055):
                        nc.sync.dma_start(
                            out=fq[:],
                            in_=f_ext[g * 4 * P:(g + 1) * 4 * P, :].rearrange(
                                "(u p) d -> p u d", p=P))
                    o_sb = opool.tile([P, 4, D], bf16, tag="o", name=f"o{g}")
                f_t = fq[:, t % 4, :]

                # snapshot prefix (tiles < t) before adding tile t, laid out
                # full-width column-disjoint: rows 32q+p hold C_p only in
                # columns [256q, 256q+256), zeros elsewhere (pre-zeroed).
                if t >= 1:
                    c_sb = c_sbs[t % 3]
                    for q in range(4):
                        src = psum_C[32 * q:32 * q + PD, 0:QD]
                        dst = c_sb[32 * q:32 * q + PD, q * QD:(q + 1) * QD]
                        if q < 2:
                            nc.vector.tensor_copy(dst, src)
                        else:
                            nc.scalar.copy(dst, src)
                    nc.vector.tensor_copy(c_sb[0:PD, D:D + 1],
                                          psum_C[0:PD, QD:QD + 1])
                else:
                    c_sb = None

                # moment accumulation for tile t (t=15 is never consumed)
                if t < NT - 1:
                    st, sp = (t == 0), (t == NT - 2)
                    for q in range(4):
                        nc.tensor.matmul(
                            psum_C[32 * q:32 * q + PD, 0:QD],
                            lhsT=kp_all[:, t, :],
                            rhs=f_t[:, q * QD:(q + 1) * QD],
                            start=st, stop=sp, skip_group_check=True,
                            tile_position=(0, 32 * q),
                        )
                    # NOTE start=False even at t=0: q0's start=True already
                    # marks this whole bank row pending-zero (2KB zero-region
                    # granularity), so H's first write overwrites correctly.
                    # start=True here would re-mark q0's freshly-written
                    # columns and lose tile 0's moments.
                    nc.tensor.matmul(
                        psum_C[0:PD, QD:QD + 1], lhsT=kp_all[:, t, :],
                        rhs=ones_col[:], start=False, stop=sp,
                        skip_group_check=True, tile_position=(0, 0),
                    )

                # ---- block t ----
                # v4[32q+p, j] = c_j^p for each group q (4 concurrent
                # col-group transposes into one bank, one copy out)
                smalls = ps_small_pool.tile([P, P + 1 + P], f32, tag="smalls",
                                            name=f"smalls{t}")
                cb = smalls[:, 0:P]
                zc = smalls[:, P:P + 1]
                vt_ps = smalls[0:3 * 32 + PD, P + 1:P + 1 + P]
                for q in range(4):
                    nc.tensor.matmul(
                        vt_ps[32 * q:32 * q + PD, :], lhsT=cp_all[:, t, :],
                        rhs=ident_bf[:], start=True, stop=True,
                        skip_group_check=True, tile_position=(0, 32 * q),
                    )
                # single [101,128] copy: gap rows pick up finite psum garbage,
                # but they multiply c_sb's zero rows in the K=128 V@C matmul
                # (rows 101-127 of v4 keep their initial memset zeros)
                v4 = v4s[t % 3]
                nc.vector.tensor_copy(v4[0:3 * 32 + PD, :],
                                      vt_ps[0:3 * 32 + PD, :])

                # cb[i, j] = c_j (broadcast along partitions)
                nc.tensor.matmul(
                    cb, lhsT=cp_all[:, t, 1:2].to_broadcast((P, P)),
                    rhs=ident_bf[:], start=True, stop=True,
                    skip_group_check=True,
                )
                # exact diagonal tile, masked upper-triangular
                s_d = sdpool.tile([P, P], bf16, tag="sd", name=f"sd{t}")
                nc.scalar.activation(s_d[:], cb, AF.Exp, scale=kT[:, t:t + 1])
                nc.gpsimd.tensor_mul(s_d[:], s_d[:], triu[:])

                # z_j = v^T H + sum_i s_d[i, j]
                if t >= 1:
                    nc.tensor.matmul(
                        zc, lhsT=v4[0:PD, :], rhs=c_sb[0:PD, D:D + 1],
                        start=True, stop=False, skip_group_check=True,
                        tile_position=(0, 0),
                    )
                nc.tensor.matmul(
                    zc, lhsT=s_d[:], rhs=ones_col[:], start=(t == 0),
                    stop=True, skip_group_check=True,
                )
                rz = rzpool.tile([P, 1], f32, tag="rz", name=f"rz{t}")
                nc.vector.reciprocal(rz[:], zc)

                # out = (s_d^T f_t + V^T C) / z
                u = t % 4
                for h in range(2):
                    out_ps = ps_out_pool.tile([P, HALF], f32, tag="out_ps",
                                              name=f"out_ps{t}_{h}")
                    nc.tensor.matmul(
                        out_ps[:], lhsT=s_d[:],
                        rhs=f_t[:, h * HALF:(h + 1) * HALF],
                        start=True, stop=(t == 0), skip_group_check=True,
                    )
                    if t >= 1:
                        # K=128 plain matmul; c_sb's column-disjoint layout
                        # (zeros off-quarter) selects the right C_p per d.
                        nc.tensor.matmul(
                            out_ps[:], lhsT=v4[:],
                            rhs=c_sb[:, h * HALF:(h + 1) * HALF],
                            start=False, stop=True, skip_group_check=True,
                        )
                    if h == 0:
                        nc.scalar.activation(o_sb[:, u, 0:HALF], out_ps[:],
                                             AF.Copy, scale=rz[:])
                    else:
                        nc.vector.tensor_scalar_mul(o_sb[:, u, HALF:D],
                                                    out_ps[:], rz[:])

                if t % 4 == 3:
                    g = t // 4
                    nc.sync.dma_start(
                        out=out_ext[g * 4 * P:(g + 1) * 4 * P, :].rearrange(
                            "(u p) d -> p u d", p=P),
                        in_=o_sb[:])

    nc.compile()
    return nc


def _get_nc():
    if "nc" not in _CACHE:
        _CACHE["nc"] = _build()
    return _CACHE["nc"]


def kernel(x, f, wk, wq, trace=False):
    from concourse.bass_utils import run_bass_kernel_spmd

    x = np.asarray(x, dtype=np.float32)
    f = np.asarray(f, dtype=np.float32)
    wk = np.asarray(wk, dtype=np.float32)
    wq = np.asarray(wq, dtype=np.float32)

    bf = ml_dtypes.bfloat16
    f8 = ml_dtypes.float8_e4m3
    xt = np.ascontiguousarray(np.transpose(x, (0, 2, 1))).astype(f8)
    fb = f.astype(bf)
    w = np.zeros((D, 32), dtype=np.float32)
    w[:, 0] = 32.0 * wk[0]
    w[:, 1] = 32.0 * wq[0]
    w8 = w.astype(f8)

    nc = _get_nc()
    in_maps = [{"xt": xt[b], "f": fb[b], "w": w8} for b in range(B)]
    res = run_bass_kernel_spmd(nc, in_maps, core_ids=list(range(B)),
                               trace=trace)
    out = np.stack(
        [res.results[b]["out"].astype(np.float32) for b in range(B)], axis=0)
    if trace:
        _CACHE["last_exec_time_ns"] = res.exec_time_ns
        _CACHE["last_results"] = res
    return out



# revision 4
# speedup vs baseline: 1.4151x; 1.4151x over previous
"""Rank-1 triangular attention via Taylor moments, v3.

Math (per batch): k = x @ wk, q = x @ wq, c = q/32; column-softmax over
i <= j of exp(k_i c_j); out[j] = sum_i a[i,j] f[i].  |k c| <= ~0.21 so
exp(k_i c_j) = sum_p (k_i^p/p!) c_j^p (5 terms, err ~3e-6).  Off-diagonal
block contributions become moment prefix sums C_p[d] = sum_i k_i^p f[i,d];
the diagonal 128x128 block uses exact exp with a triangular mask.

v3 changes vs v2 (v2 ran ~100-134us, PE HAM-throttled to 1.2 GHz with
~2us/tile PE stalls and ~2.9us/tile of small DVE/ACT copies):
  - moments accumulate as TWO 512-col halves plus an H strip in one psum
    bank (PE col strips 0-4 / 32-36 / 64-68 run concurrently); snapshots
    shrink from 5 copies to 2x[5,512]+[5,1] into a flat c_sb [5,1025].
  - V^T C becomes two K=5 matmuls (lhsT = cpT [5,128]) against the flat
    c_sb -- the 101-row zero-padded v4, its big copy, and the 4 col-group
    transposes are gone (one [5,128] transpose instead).
  - stage A k/q uses fp8 DoubleRow matmuls (K=256/instr, 2 cols/cycle)
    with M=2 output partitions, fed by a host-packed pair layout.
  - the loop is software-pipelined one tile ahead: cb/exp/mask/cpT for
    tile t+1 are emitted while tile t's big matmuls run, so the PE never
    waits on ACT/GpSimd and stays HAM-warm at 2.4 GHz.
"""

import sys

sys.path.insert(0, "/opt/trn_rl_repo")

import numpy as np
import ml_dtypes

B, N, D = 8, 2048, 1024
P = 128
NT = N // P       # 16 i/j tiles
PD = 5            # Taylor terms p=0..4
HALF = D // 2     # 512
NB = N // 256     # 8 stage-A n-blocks

_CACHE = {}


def _patch_compiler_flags():
    from concourse import compiler_utils as cu

    flags = [f.replace("--enable-ldw-opt=false", "--enable-ldw-opt=true")
             for f in cu.get_compiler_flags()]
    cu.set_compiler_flags(flags)


def _build():
    import concourse.bacc as bacc
    import concourse.mybir as mybir
    from concourse.tile import TileContext
    from concourse.masks import make_identity, make_upper_triangular

    _patch_compiler_flags()

    dt = mybir.dt
    f32 = dt.float32
    bf16 = dt.bfloat16
    fp8 = dt.float8e4
    AF = mybir.ActivationFunctionType
    ALU = mybir.AluOpType
    DR = mybir.MatmulPerfMode.DoubleRow

    nc = bacc.Bacc(None, target_bir_lowering=False)
    # xt: [p, (c,i,n)] with d = (2c+i)*128+p, host-packed for DoubleRow
    xt_ext = nc.declare_dram_parameter("xt", [P, 4 * 2 * N], fp8,
                                       isOutput=False)
    f_ext = nc.declare_dram_parameter("f", [N, D], bf16, isOutput=False)
    # w: [p, (c,i,m)] m in {32k, 32q}
    w_ext = nc.declare_dram_parameter("w", [P, 4 * 2 * 16], fp8,
                                      isOutput=False)
    out_ext = nc.declare_dram_parameter("out", [N, D], bf16, isOutput=True)

    with TileContext(nc) as tc:
        with (
            tc.tile_pool(name="const", bufs=1) as cpool,
            tc.tile_pool(name="fin", bufs=3) as fpool,
            tc.tile_pool(name="sd", bufs=3) as sdpool,
            tc.tile_pool(name="outsb", bufs=2) as opool,
            tc.tile_pool(name="csb", bufs=3) as cspool,
            tc.tile_pool(name="v4p", bufs=3) as v4pool,
            tc.tile_pool(name="rz", bufs=3) as rzpool,
            tc.tile_pool(name="ps_C", bufs=1, space="PSUM") as ps_C_pool,
            tc.tile_pool(name="ps_out", bufs=4, space="PSUM") as ps_out_pool,
            tc.tile_pool(name="ps_small", bufs=3, space="PSUM") as ps_small_pool,
        ):
            # ---------------- constants ----------------
            ident_bf = cpool.tile([P, P], bf16, tag="ident_bf")
            make_identity(nc, ident_bf[:])
            triu = cpool.tile([P, P], bf16, tag="triu")
            make_upper_triangular(nc, triu[:], val=1.0, diag=True)
            ones_col = cpool.tile([P, 1], bf16, tag="ones_col")
            nc.gpsimd.memset(ones_col[:], 1.0)

            # persistent moment accumulator, one bank:
            # rows 0-4  cols 0:512 = C_p for d in [0,512)
            # rows 32-36 cols 0:512 = C_p for d in [512,1024)
            # rows 64-68 col 0      = H_p = sum_i k_i^p
            psum_C = ps_C_pool.tile([3 * 32 + PD, HALF], f32, tag="psC",
                                    name="psC")

            # PE warm-up: garbage matmuls release the HAM clock-gate while
            # the xt DMA streams in.
            warm_ps = ps_out_pool.tile([P, HALF], f32, tag="out_ps",
                                       name="warm")
            for _ in range(24):
                nc.tensor.matmul(
                    warm_ps[:, 0:P], lhsT=ident_bf[:], rhs=triu[:],
                    start=True, stop=True, skip_group_check=True,
                )

            # ---------------- stage A: k,q ----------------
            w_sb = cpool.tile([P, 4, 2, 16], fp8, tag="w_sb")
            nc.sync.dma_start(
                out=w_sb[:], in_=w_ext.rearrange("p (c i m) -> p c i m",
                                                 c=4, i=2))
            xt_sb = cpool.tile([P, 4, 2, N], fp8, tag="xt_sb")
            for c in range(4):
                nc.sync.dma_start(
                    out=xt_sb[:, c, :, :],
                    in_=xt_ext[:, c * 2 * N:(c + 1) * 2 * N].rearrange(
                        "p (i n) -> p i n", i=2),
                )

            # kq_sb rows: 0 = 32k, 1 = 32q (host pre-scales w by 32)
            kq_sb = cpool.tile([2, N], bf16, tag="kq_sb")
            for nb in range(NB):
                kq_ps = ps_small_pool.tile([16, 256], f32, tag="smalls",
                                           name=f"kq_ps{nb}")
                for c in range(4):
                    nc.tensor.matmul(
                        kq_ps[:], lhsT=w_sb[:, c, :, :],
                        rhs=xt_sb[:, c, :, nb * 256:(nb + 1) * 256],
                        start=(c == 0), stop=(c == 3), perf_mode=DR,
                    )
                dst = kq_sb[:, nb * 256:(nb + 1) * 256]
                if nb % 2 == 0:
                    nc.vector.tensor_copy(dst, kq_ps[0:2, :])
                else:
                    nc.scalar.copy(dst, kq_ps[0:2, :])

            # per-tile k/c columns: PE transpose of [2,128] slices, batched
            # into one psum tile then one copy
            kc_ps = ps_small_pool.tile([P, 2 * NT], f32, tag="smalls",
                                       name="kc_ps")
            for t in range(NT):
                nc.tensor.matmul(
                    kc_ps[:, 2 * t:2 * t + 2],
                    lhsT=kq_sb[:, t * P:(t + 1) * P],
                    rhs=ident_bf[0:2, 0:2], start=True, stop=True,
                    skip_group_check=True,
                )
            kc_all = cpool.tile([P, NT, 2], f32, tag="kc_all")
            nc.vector.tensor_copy(
                kc_all[:], kc_ps[:].rearrange("p (t c) -> p t c", c=2))

            # bulk power tables over all tiles at once ([128,16] ops)
            kT = cpool.tile([P, NT], f32, tag="kT")   # true k (for exp scale)
            nc.vector.tensor_scalar_mul(kT[:], kc_all[:, :, 0], 1.0 / 32.0)
            kp_all = cpool.tile([P, NT, PD], bf16, tag="kp_all")
            cp_all = cpool.tile([P, NT, PD], bf16, tag="cp_all")
            nc.gpsimd.memset(kp_all[:, :, 0:1], 1.0)
            nc.gpsimd.memset(cp_all[:, :, 0:1], 1.0)
            nc.vector.tensor_copy(kp_all[:, :, 1], kT[:])
            nc.vector.tensor_scalar_mul(cp_all[:, :, 1], kc_all[:, :, 1],
                                        1.0 / 1024.0)
            # kp: k^p/p! ; cp: c^p (factorials folded into the k side)
            nc.vector.scalar_tensor_tensor(
                out=kp_all[:, :, 2], in0=kp_all[:, :, 1], scalar=0.5,
                in1=kp_all[:, :, 1], op0=ALU.mult, op1=ALU.mult)
            nc.vector.scalar_tensor_tensor(
                out=kp_all[:, :, 3], in0=kp_all[:, :, 2], scalar=1.0 / 3.0,
                in1=kp_all[:, :, 1], op0=ALU.mult, op1=ALU.mult)
            nc.vector.scalar_tensor_tensor(
                out=kp_all[:, :, 4], in0=kp_all[:, :, 3], scalar=0.25,
                in1=kp_all[:, :, 1], op0=ALU.mult, op1=ALU.mult)
            nc.vector.scalar_tensor_tensor(
                out=cp_all[:, :, 2], in0=cp_all[:, :, 1], scalar=1.0,
                in1=cp_all[:, :, 1], op0=ALU.mult, op1=ALU.mult)
            nc.vector.scalar_tensor_tensor(
                out=cp_all[:, :, 3], in0=cp_all[:, :, 2], scalar=1.0,
                in1=cp_all[:, :, 1], op0=ALU.mult, op1=ALU.mult)
            nc.vector.scalar_tensor_tensor(
                out=cp_all[:, :, 4], in0=cp_all[:, :, 3], scalar=1.0,
                in1=cp_all[:, :, 1], op0=ALU.mult, op1=ALU.mult)

            # ---------------- pipelined prologue for tile 0 ----------------
            def emit_sd(t):
                """cb -> exp -> triu mask for tile t; returns s_d tile."""
                cb_ps = ps_small_pool.tile([P, P], f32, tag="smalls",
                                           name=f"cb{t}")
                nc.tensor.matmul(
                    cb_ps[:], lhsT=cp_all[:, t, 1:2].to_broadcast((P, P)),
                    rhs=ident_bf[:], start=True, stop=True,
                    skip_group_check=True,
                )
                s_d = sdpool.tile([P, P], bf16, tag="sd", name=f"sd{t}")
                nc.scalar.activation(s_d[:], cb_ps[:], AF.Exp,
                                     scale=kT[:, t:t + 1])
                nc.gpsimd.tensor_mul(s_d[:], s_d[:], triu[:])
                return s_d

            def emit_v4(t):
                """cpT transpose + copy for tile t; returns v4 [5,128]."""
                vt_ps = ps_small_pool.tile([PD, P], f32, tag="smalls",
                                           name=f"vt{t}")
                nc.tensor.matmul(
                    vt_ps[:], lhsT=cp_all[:, t, :], rhs=ident_bf[:],
                    start=True, stop=True, skip_group_check=True,
                )
                v4 = v4pool.tile([PD, P], bf16, tag="v4", name=f"v4{t}")
                nc.vector.tensor_copy(v4[:], vt_ps[:])
                return v4

            s_d_next = emit_sd(0)
            v4_next = emit_v4(0)

            # ---------------- main loop ----------------
            fq = None
            o_sb = None
            for t in range(NT):
                if t % 4 == 0:
                    g = t // 4
                    fq = fpool.tile([P, 4, D], bf16, tag="fq", name=f"fq{g}")
                    # floor f loads so the xt load (which gates all compute)
                    # gets HBM bandwidth first
                    with tc.tile_wait_until(0.004):
                        nc.sync.dma_start(
                            out=fq[:],
                            in_=f_ext[g * 4 * P:(g + 1) * 4 * P, :].rearrange(
                                "(u p) d -> p u d", p=P))
                    o_sb = opool.tile([P, 4, D], bf16, tag="o", name=f"o{g}")
                f_t = fq[:, t % 4, :]
                s_d = s_d_next
                v4 = v4_next

                # pipeline tile t+1's ACT/GpSimd work behind tile t's matmuls
                if t + 1 < NT:
                    s_d_next = emit_sd(t + 1)
                    v4_next = emit_v4(t + 1)

                # snapshot prefix (tiles < t) before adding tile t:
                # c_sb = [C(d<512) 512 | C(d>=512) 512 | H 1]
                if t >= 1:
                    c_sb = cspool.tile([PD, D + 1], bf16, tag="csb",
                                       name=f"csb{t}")
                    nc.scalar.copy(c_sb[:, 0:HALF], psum_C[0:PD, :])
                    nc.vector.tensor_copy(c_sb[:, HALF:D],
                                          psum_C[32:32 + PD, :])
                    nc.vector.tensor_copy(c_sb[:, D:D + 1],
                                          psum_C[64:64 + PD, 0:1])
                else:
                    c_sb = None

                # out = (s_d^T f_t + cpT^T C) / z
                u = t % 4
                out_pss = []
                for h in range(2):
                    out_ps = ps_out_pool.tile([P, HALF], f32, tag="out_ps",
                                              name=f"out_ps{t}_{h}")
                    nc.tensor.matmul(
                        out_ps[:], lhsT=s_d[:],
                        rhs=f_t[:, h * HALF:(h + 1) * HALF],
                        start=True, stop=(t == 0), skip_group_check=True,
                    )
                    if t >= 1:
                        nc.tensor.matmul(
                            out_ps[:], lhsT=v4[:],
                            rhs=c_sb[:, h * HALF:(h + 1) * HALF],
                            start=False, stop=True, skip_group_check=True,
                        )
                    out_pss.append(out_ps)

                # z_j = sum_i s_d[i,j] + sum_p c_j^p H_p
                zc_ps = ps_small_pool.tile([P, 2], f32, tag="smalls",
                                           name=f"zc{t}")
                zc = zc_ps[:, 0:1]
                nc.tensor.matmul(
                    zc, lhsT=s_d[:], rhs=ones_col[:], start=True,
                    stop=(t == 0), skip_group_check=True,
                )
                if t >= 1:
                    nc.tensor.matmul(
                        zc, lhsT=v4[:], rhs=c_sb[:, D:D + 1],
                        start=False, stop=True, skip_group_check=True,
                    )
                rz = rzpool.tile([P, 1], f32, tag="rz", name=f"rz{t}")
                nc.vector.reciprocal(rz[:], zc)

                # moment accumulation for tile t (t=15 is never consumed);
                # must come after the snapshot reads of prefix < t
                if t < NT - 1:
                    st, sp = (t == 0), (t == NT - 2)
                    for h in range(2):
                        nc.tensor.matmul(
                            psum_C[32 * h:32 * h + PD, :],
                            lhsT=kp_all[:, t, :],
                            rhs=f_t[:, h * HALF:(h + 1) * HALF],
                            start=st, stop=sp, skip_group_check=True,
                            tile_position=(0, 32 * h),
                        )
                    nc.tensor.matmul(
                        psum_C[64:64 + PD, 0:1], lhsT=kp_all[:, t, :],
                        rhs=ones_col[:], start=st, stop=sp,
                        skip_group_check=True, tile_position=(0, 64),
                    )

                nc.scalar.activation(o_sb[:, u, 0:HALF], out_pss[0][:],
                                     AF.Copy, scale=rz[:])
                nc.vector.tensor_scalar_mul(o_sb[:, u, HALF:D],
                                            out_pss[1][:], rz[:])

                if t % 4 == 3:
                    g = t // 4
                    nc.sync.dma_start(
                        out=out_ext[g * 4 * P:(g + 1) * 4 * P, :].rearrange(
                            "(u p) d -> p u d", p=P),
                        in_=o_sb[:])

    nc.compile()
    return nc


def _get_nc():
    if "nc" not in _CACHE:
        _CACHE["nc"] = _build()
    return _CACHE["nc"]


def kernel(x, f, wk, wq, trace=False):
    from concourse.bass_utils import run_bass_kernel_spmd

    x = np.asarray(x, dtype=np.float32)
    f = np.asarray(f, dtype=np.float32)
    wk = np.asarray(wk, dtype=np.float32)
    wq = np.asarray(wq, dtype=np.float32)

    bf = ml_dtypes.bfloat16
    f8 = ml_dtypes.float8_e4m3
    # xt pair layout: xt_p[b][p, c, i, n] = x[b, n, (2c+i)*128+p]
    xt = np.ascontiguousarray(np.transpose(x, (0, 2, 1)))  # [B, D, N]
    xt_p = xt.reshape(B, 4, 2, P, N).transpose(0, 3, 1, 2, 4)
    xt_p = np.ascontiguousarray(xt_p.reshape(B, P, 4 * 2 * N)).astype(f8)
    fb = f.astype(bf)
    w = np.zeros((D, 16), dtype=np.float32)
    w[:, 0] = 32.0 * wk[0]
    w[:, 1] = 32.0 * wq[0]
    w_p = w.reshape(4, 2, P, 16).transpose(2, 0, 1, 3)
    w_p = np.ascontiguousarray(w_p.reshape(P, 4 * 2 * 16)).astype(f8)

    nc = _get_nc()
    in_maps = [{"xt": xt_p[b], "f": fb[b], "w": w_p} for b in range(B)]
    res = run_bass_kernel_spmd(nc, in_maps, core_ids=list(range(B)),
                               trace=trace)
    out = np.stack(
        [res.results[b]["out"].astype(np.float32) for b in range(B)], axis=0)
    if trace:
        _CACHE["last_exec_time_ns"] = res.exec_time_ns
        _CACHE["last_results"] = res
    return out
